# revision 1
# baseline (speedup 1.0000x reference)
"""Dense CRF pairwise loss on 8 Trainium2 NeuronCores (upper-triangle scheme).

loss = (2/N) * [ sum_{i<j} (a_i b_j + a_j b_i) K_ij + sum_i a_i b_i ],
a = probs[:,0], b = 1-a, K_ij = exp(-c1*d_xy - c2*d_rgb) (symmetric, K_ii=1).
The diagonal term is computed on host; the triangle sum on-device.

Exponent as one bf16 matmul (contraction 26) built from exactly-representable
pieces: positions are small ints (exact in bf16); 120*rgb and the per-pixel
base term are hi/mid/lo bf16 splits; products below ~2^-22 relative dropped.
ScalarE applies exp with its free scale=-c1, writing K in bf16.

Per unit ([128 i] x [512 j] block): one more matmul with stationary [128,4] =
[a_hi a_lo b_hi b_lo] reduces over i, PSUM-accumulated per j-column; one DVE
tensor_tensor_reduce per column dots the 4 rows with [b b a a] weights.

SPMD uniformity: every core runs an identical 90-slot schedule (ceil((m+1)/2)
slots per column m). Which i-tile a slot handles is pure per-core DATA: cores
0-3 always take the 4 diagonal (masked) blocks of each column in slot 0 and
carry a single triangular 0/1 mask pattern; cores 4-7 carry an all-ones mask;
leftover slots get zero features (exp -> 1, but stat rows are 0 -> no effect).
"""

import numpy as np
import ml_dtypes

import concourse.bass as bass
import concourse.tile as tile
from concourse import bacc, mybir
from concourse.bass_utils import run_bass_kernel_spmd

BF = ml_dtypes.bfloat16

H = W = 96
N = H * W                      # 9216
N_CORES = 8
JC = 512                       # j-chunk (column) width
N_COLS = N // JC               # 18
IT = 128                       # i-tile height
KDIM = 26
EGROUP = 3                     # units per ScalarE exp instruction (3 psum banks)

SIGMA_XY = 15.0
SIGMA_RGB = 0.125
C1 = 1.0 / (2.0 * SIGMA_XY * SIGMA_XY)
C2 = 1.0 / (2.0 * SIGMA_RGB * SIGMA_RGB)
LAM = 120.0                    # sqrt(C2/C1)

# slots per column m (uniform across cores)
CSLOTS = [(m + 2) // 2 for m in range(N_COLS)]   # ceil((m+1)/2)
NSLOTS = sum(CSLOTS)                             # 90
# natural order except the single-slot column 0 goes last: the final
# column's ACT->mask->r->copy chain is the kernel tail, keep it short
COL_ORDER = list(range(1, N_COLS)) + [0]

_CACHE = {}


def _slot_map(core):
    """slot -> i-tile index t, or -1 for dummy. Column m has units t=0..4m+3;
    t=4m+c goes to core c (c<4) slot 0 (masked); unmasked t<4m fill the rest."""
    out = []
    for m in COL_ORDER:
        ns = CSLOTS[m]
        for s in range(ns):
            if s == 0:
                t = (4 * m + core) if core < 4 else (core - 4 if core - 4 < 4 * m else -1)
            else:
                idx = 4 + (s - 1) * 8 + core
                t = idx if idx < 4 * m else -1
            out.append(t)
    return out


def _build_program():
    nc = bacc.Bacc("TRN2", target_bir_lowering=False, debug=False)
    f32 = mybir.dt.float32
    b16 = mybir.dt.bfloat16

    uf_d = nc.dram_tensor("uf", [KDIM, NSLOTS * IT], b16, kind="ExternalInput")
    vf_d = nc.dram_tensor("vf", [KDIM, N], b16, kind="ExternalInput")
    st_d = nc.dram_tensor("st", [128, NSLOTS * 4], b16, kind="ExternalInput")
    mk_d = nc.dram_tensor("mk", [128, JC], b16, kind="ExternalInput")
    stage_d = nc.dram_tensor("stage", [4, N], f32, kind="ExternalOutput")

    # flat slot list with (column m, s, global slot index)
    slots = []
    for m in COL_ORDER:
        for s in range(CSLOTS[m]):
            slots.append((m, s))
    groups = [slots[i:i + EGROUP] for i in range(0, NSLOTS, EGROUP)]

    with tile.TileContext(nc) as tc:
        with (
            tc.tile_pool(name="const", bufs=1) as cpool,
            tc.tile_pool(name="kgrp", bufs=4) as kpool,
            tc.tile_pool(name="pse", bufs=2, space="PSUM") as pe_pool,
            tc.tile_pool(name="psr", bufs=2, space="PSUM") as pr_pool,
        ):
            uf_t = cpool.tile([KDIM, NSLOTS * IT], b16)
            vf_t = cpool.tile([KDIM, N], b16)
            st_t = cpool.tile([128, NSLOTS * 4], b16)
            mk_t = cpool.tile([128, JC], b16)
            stage_t = cpool.tile([4, N], f32)
            # split input DMAs across two queues, first-used pieces first
            DMA_SPLIT = 4
            for q in range(DMA_SPLIT):
                vs = (N // DMA_SPLIT)
                nc.sync.dma_start(vf_t[:, q * vs:(q + 1) * vs],
                                  vf_d.ap()[:, q * vs:(q + 1) * vs])
                us = (NSLOTS * IT // DMA_SPLIT)
                nc.gpsimd.dma_start(uf_t[:, q * us:(q + 1) * us],
                                    uf_d.ap()[:, q * us:(q + 1) * us])
            nc.sync.dma_start(st_t[:], st_d.ap())
            nc.sync.dma_start(mk_t[:], mk_d.ap())

            # HAM warm-up burst: dense K=128 matmuls while DMAs land.
            warm_t = cpool.tile([128, 512], b16)
            nc.gpsimd.memset(warm_t[:], 0.0)
            warm_ps = pr_pool.tile([128, 512], mybir.dt.float32, tag="psr",
                                   name="warm_ps")
            for _ in range(14):
                nc.tensor.matmul(warm_ps[:], warm_t[:, :128], warm_t[:],
                                 start=True, stop=True)

            # pipeline with a small group delay between E/exp and the
            # r-matmuls so the PE never sits on r-mms ahead of E-group fills.
            pending = []          # (ktile, [(m, s, slot_idx, pos), ...])
            r_tiles = {}          # column m -> psum r tile
            done_cols = [0]
            dma_marks = {6: None, 12: None}

            def emit_r(ktile, infos):
                for (m, s, gslot, pos) in infos:
                    if s == 0:
                        nc.vector.tensor_mul(
                            ktile[:, pos * JC:(pos + 1) * JC],
                            ktile[:, pos * JC:(pos + 1) * JC],
                            mk_t[:],
                        )
                    if m not in r_tiles:
                        r_tiles[m] = pr_pool.tile([128, JC], mybir.dt.float32,
                                                  tag="psr", name=f"psr{m}")
                    nc.tensor.matmul(
                        r_tiles[m][0:4, :],
                        st_t[:, gslot * 4:(gslot + 1) * 4],
                        ktile[:, pos * JC:(pos + 1) * JC],
                        start=(s == 0),
                        stop=(s == CSLOTS[m] - 1),
                    )
                    if s == CSLOTS[m] - 1:
                        nc.vector.tensor_copy(
                            stage_t[:, m * JC:(m + 1) * JC],
                            r_tiles[m][0:4, :],
                        )
                        done_cols[0] += 1
                        # COL_ORDER = [1..17, 0]: completed cols are contiguous
                        if done_cols[0] == 6:
                            nc.sync.dma_start(stage_d.ap()[:, 1 * JC:7 * JC],
                                              stage_t[:, 1 * JC:7 * JC])
                        elif done_cols[0] == 12:
                            nc.sync.dma_start(stage_d.ap()[:, 7 * JC:13 * JC],
                                              stage_t[:, 7 * JC:13 * JC])
                        elif done_cols[0] == 17:
                            nc.sync.dma_start(stage_d.ap()[:, 13 * JC:],
                                              stage_t[:, 13 * JC:])

            gslot = 0
            for grp in groups:
                ps = pe_pool.tile([128, EGROUP * JC], mybir.dt.float32, tag="pse")
                infos = []
                for pos, (m, s) in enumerate(grp):
                    nc.tensor.matmul(
                        ps[:, pos * JC:(pos + 1) * JC],
                        uf_t[:, gslot * IT:(gslot + 1) * IT],
                        vf_t[:, m * JC:(m + 1) * JC],
                        start=True,
                        stop=True,
                    )
                    infos.append((m, s, gslot, pos))
                    gslot += 1
                ktile = kpool.tile([128, EGROUP * JC], b16, tag="kg")
                nc.scalar.activation(
                    ktile[:], ps[:],
                    mybir.ActivationFunctionType.Exp,
                    scale=float(-C1),
                )
                pending.append((ktile, infos))
                if len(pending) > 1:
                    emit_r(*pending.pop(0))
            while pending:
                emit_r(*pending.pop(0))

            nc.sync.dma_start(stage_d.ap()[:, 0:JC], stage_t[:, 0:JC])

    nc.compile()
    return nc


def _split3(x):
    h = x.astype(BF)
    r = x - h.astype(np.float64)
    m = r.astype(BF)
    l = (r - m.astype(np.float64)).astype(BF)
    return h, m, l


def _features(probs, image):
    ys, xs = np.meshgrid(np.arange(H, dtype=np.float64),
                         np.arange(W, dtype=np.float64), indexing="ij")
    y = ys.ravel()
    x = xs.ravel()
    col = image[0].astype(np.float64).reshape(3, N)
    a = probs[0, 0].astype(np.float64).reshape(N)
    b = 1.0 - a

    g = LAM * col
    base = y * y + x * x + (g * g).sum(axis=0)
    A1, A2, A3 = _split3(base)
    gh, gm, gl = _split3(g)

    one = np.ones(N, BF)
    u_rows = [A1, A2, A3, one, one, one,
              (-2.0 * y).astype(BF), (-2.0 * x).astype(BF)]
    v_rows = [one, one, one, A1, A2, A3, y.astype(BF), x.astype(BF)]
    for ch in range(3):
        h64 = gh[ch].astype(np.float64)
        m64 = gm[ch].astype(np.float64)
        l64 = gl[ch].astype(np.float64)
        n2 = lambda t: (-2.0 * t).astype(BF)
        u_rows += [n2(h64), n2(h64), n2(m64), n2(h64), n2(l64), n2(m64)]
        v_rows += [gh[ch], gm[ch], gh[ch], gl[ch], gh[ch], gm[ch]]
    u = np.stack(u_rows).astype(BF)     # [26, N] stationary (i side)
    v = np.stack(v_rows).astype(BF)     # [26, N] moving (j side)

    ah = a.astype(BF)
    al = (a - ah.astype(np.float64)).astype(BF)
    bh = b.astype(BF)
    bl = (b - bh.astype(np.float64)).astype(BF)
    stat = np.stack([ah, al, bh, bl], axis=1)      # [N, 4]

    diag = float((a * b).sum())
    return u, v, stat, a, b, diag


def kernel(probs: np.ndarray, image: np.ndarray) -> np.ndarray:
    probs = np.asarray(probs)
    image = np.asarray(image)
    assert probs.shape == (1, 2, H, W) and image.shape == (1, 3, H, W)

    if "nc" not in _CACHE:
        _CACHE["nc"] = _build_program()
    nc = _CACHE["nc"]

    u, v, stat, a, b, diag = _features(probs, image)

    p = np.arange(128)[:, None]
    f = np.arange(JC)[None, :]
    in_maps = []
    for c in range(N_CORES):
        smap = _slot_map(c)
        uf = np.zeros((KDIM, NSLOTS * IT), dtype=BF)
        st = np.zeros((128, NSLOTS * 4), dtype=BF)
        for slot, t in enumerate(smap):
            if t < 0:
                continue
            uf[:, slot * IT:(slot + 1) * IT] = u[:, t * IT:(t + 1) * IT]
            st[:, slot * 4:(slot + 1) * 4] = stat[t * IT:(t + 1) * IT, :]
        if c < 4:
            mk = (f > 128 * c + p).astype(BF)
        else:
            mk = np.ones((128, JC), dtype=BF)
        in_maps.append({"uf": uf, "vf": v, "st": st, "mk": mk})
    _CACHE["in_maps"] = in_maps

    res = run_bass_kernel_spmd(nc, in_maps, list(range(N_CORES)))
    tri = np.float64(0.0)
    for c in range(N_CORES):
        stage = res.results[c]["stage"].astype(np.float64)  # [4, N]
        tri += ((stage[0] + stage[1]) * b).sum() + ((stage[2] + stage[3]) * a).sum()

    loss = 2.0 * (tri + diag) / N
    return np.float32(loss)



# revision 8
# speedup vs baseline: 1.0134x; 1.0134x over previous
"""Dense CRF pairwise loss on 8 Trainium2 NeuronCores.

loss = (2/N) * [ sum_{i<j} (a_i b_j + a_j b_i) K_ij + sum_i a_i b_i ],
a = probs[:,0], b = 1-a, K_ij = exp(-c1*d_xy - c2*d_rgb), K_ii = 1.
Diagonal term on host; strict-upper-triangle sum on device.

Pixels are permuted into 8x16 patches (i-tiles of 128) grouped into 16x32
chunks (j-columns of 512).  Blocks whose patch boxes are farther than
RCUT=33 px apart are culled (the Gaussian tail there is negligible):
59 slots/core instead of 90.

The exponent x = c1*d_xy + c2*d_rgb is ONE fp8e4m3 DoubleRow matmul
(rank 36 packed as [18 partitions x 2 k-tiles], 0.5 cy/col): features are
pre-scaled by sqrt(c1) and hi/mid/lo-split so every value is e4m3-exact;
products are exact in the PE.  ScalarE applies exp(-x + ln 128) writing
K~ = 128*K in fp8 (the 2^7 bias preserves small-K mass vs e4m3 flush).

Per column the per-slot stats [ah al bh bl] (fp8) reduce K~ over i with
DoubleRow r-matmuls, two slots per matmul ([128,2,4]^T @ [128,2,512]);
odd tails use a plain fp8 matmul.  The column result [4,512] is dotted
with [b,b,a,a] (bf16) on DVE (tensor_tensor_reduce) into stage[4,18] --
the output DMA is 288 bytes.

SPMD: all cores run the identical 59-slot schedule; slot->i-tile is
per-core data.  Slot 0 of every column holds the diagonal-ish blocks on
cores 0-3 with a triangular mask built on-device (iota + is_gt against a
per-core threshold; cores 4-7 get an always-true threshold = all-ones).
Padding slots have zero features and zero stats (exp -> 128, stats 0).
"""

import numpy as np
import ml_dtypes

import concourse.bass as bass
import concourse.tile as tile
from concourse import bacc, mybir
from concourse.bass_utils import run_bass_kernel_spmd

E4 = ml_dtypes.float8_e4m3
BF = ml_dtypes.bfloat16

H = W = 96
N = H * W                       # 9216
N_CORES = 8
JC = 512                        # column width (one 16x32 chunk)
N_COLS = 18
IT = 128                        # i-tile (8x16 patch)
RHALF = 18                      # contraction partitions; rank = 2*18 = 36
RCUT = 33.0                     # patch-box cull radius (px)
KSCALE = 128.0                  # K stored as 128*K in fp8
LN_KSCALE = float(np.log(KSCALE))

SIGMA_XY = 15.0
SIGMA_RGB = 0.125
C1 = 1.0 / (2.0 * SIGMA_XY * SIGMA_XY)
C2 = 1.0 / (2.0 * SIGMA_RGB * SIGMA_RGB)
LAM = np.sqrt(C2 / C1)          # 120

_CACHE = {}


# ---------------- geometry: patches, chunks, cull, schedule ----------------

def _geometry():
    boxes = []          # per patch (y0,y1,x0,x1) inclusive
    perm = []           # new pixel index -> original row-major index
    for cy in range(6):
        for cx in range(3):
            for py in range(2):
                for px in range(2):
                    y0, x0 = cy * 16 + py * 8, cx * 32 + px * 16
                    boxes.append((y0, y0 + 7, x0, x0 + 15))
                    for yy in range(y0, y0 + 8):
                        for xx in range(x0, x0 + 16):
                            perm.append(yy * 96 + xx)
    perm = np.array(perm)
    cbox = [(min(boxes[4 * m + q][0] for q in range(4)),
             max(boxes[4 * m + q][1] for q in range(4)),
             min(boxes[4 * m + q][2] for q in range(4)),
             max(boxes[4 * m + q][3] for q in range(4))) for m in range(N_COLS)]

    def bdist(b1, b2):
        dy = max(0, b1[0] - b2[1], b2[0] - b1[1])
        dx = max(0, b1[2] - b2[3], b2[2] - b1[3])
        return float(np.hypot(dy, dx))

    kept = {m: [t for t in range(4 * m + 4)
                if bdist(boxes[t], cbox[m]) <= RCUT] for m in range(N_COLS)}
    s_m = {m: max(1, -(-len(kept[m]) // N_CORES)) for m in range(N_COLS)}
    # big columns first (DMA streaming + short tail), 1-slot columns last
    col_order = sorted(range(N_COLS), key=lambda m: -s_m[m])
    s_o = [s_m[m] for m in col_order]

    assign = {}
    for m in range(N_COLS):
        A = -np.ones((N_CORES, s_m[m]), int)
        for c in range(4):
            A[c, 0] = 4 * m + c
        rest = [t for t in kept[m] if t < 4 * m]
        pos = [(c, s) for s in range(s_m[m]) for c in range(N_CORES)
               if not (s == 0 and c < 4)]
        for k, t in enumerate(rest):
            A[pos[k]] = t
        assign[m] = A
    return perm, col_order, s_o, assign


PERM, COL_ORDER, S_O, ASSIGN = _geometry()
NSLOTS = sum(S_O)               # 59


# ---------------- device program ----------------

def _build_program():
    nc = bacc.Bacc("TRN2", target_bir_lowering=False, debug=False)
    f32 = mybir.dt.float32
    i32 = mybir.dt.int32
    fp8 = mybir.dt.float8e4
    b16 = mybir.dt.bfloat16
    DR = mybir.MatmulPerfMode.DoubleRow

    uf_d = nc.dram_tensor("uf", [RHALF, 2, NSLOTS * IT], fp8, kind="ExternalInput")
    vf_d = nc.dram_tensor("vf", [RHALF, 2, N], fp8, kind="ExternalInput")
    st_d = nc.dram_tensor("st", [128, NSLOTS, 16], fp8, kind="ExternalInput")
    th_d = nc.dram_tensor("th", [128, 1], f32, kind="ExternalInput")
    wj_d = nc.dram_tensor("wj", [4, N], b16, kind="ExternalInput")
    stage_d = nc.dram_tensor("stage", [4, N_COLS], f32, kind="ExternalOutput")

    # slot base per ordered column
    base_o = np.concatenate([[0], np.cumsum(S_O)]).astype(int)

    with tile.TileContext(nc) as tc:
        with (
            tc.tile_pool(name="const", bufs=1) as cpool,
            tc.tile_pool(name="kgrp", bufs=2) as kpool,
            tc.tile_pool(name="rsc", bufs=2) as rpool,
            tc.tile_pool(name="pse", bufs=2, space="PSUM") as pe_pool,
            tc.tile_pool(name="psr", bufs=2, space="PSUM") as pr_pool,
        ):
            uf_t = cpool.tile([RHALF, 2, NSLOTS * IT], fp8)
            vf_t = cpool.tile([RHALF, 2, N], fp8)
            st_t = cpool.tile([128, NSLOTS, 16], fp8)
            th_t = cpool.tile([128, 1], f32)
            wj_t = cpool.tile([4, N], b16)
            iota_t = cpool.tile([128, JC], i32)
            mk_t = cpool.tile([128, JC], fp8)
            stage_t = cpool.tile([4, N_COLS], f32)
            warm_t = cpool.tile([RHALF, 2, JC], fp8)
            bias_t = cpool.tile([128, 1], f32)

            # --- input DMAs (triggers allowed on sync/gpsimd/scalar only)
            # gpsimd: mask prereqs, then uf in slot order
            nc.gpsimd.dma_start(th_t[:], th_d.ap())
            nc.gpsimd.iota(iota_t[:], pattern=[[1, JC]], base=0,
                           channel_multiplier=0)
            nc.gpsimd.tensor_scalar(mk_t[:], iota_t[:], th_t[:, 0:1], None,
                                    mybir.AluOpType.is_gt)
            nc.gpsimd.dma_start(uf_t[:, :, 0:10 * IT], uf_d.ap()[:, :, 0:10 * IT])
            nc.gpsimd.dma_start(uf_t[:, :, 10 * IT:32 * IT],
                                uf_d.ap()[:, :, 10 * IT:32 * IT])
            nc.gpsimd.dma_start(uf_t[:, :, 32 * IT:], uf_d.ap()[:, :, 32 * IT:])
            # sync: stats, vf in column order
            nc.sync.dma_start(st_t[:], st_d.ap())
            nc.sync.dma_start(vf_t[:, :, 0:3 * JC], vf_d.ap()[:, :, 0:3 * JC])
            nc.sync.dma_start(vf_t[:, :, 3 * JC:10 * JC],
                              vf_d.ap()[:, :, 3 * JC:10 * JC])
            # scalar: weights early (first column reduce needs them), late vf
            nc.scalar.dma_start(wj_t[:], wj_d.ap())
            nc.scalar.dma_start(vf_t[:, :, 10 * JC:], vf_d.ap()[:, :, 10 * JC:])
            # vector builds the warm-up zeros (engine op, not a DMA)
            nc.vector.memset(warm_t[:], 0.0)
            nc.vector.memset(bias_t[:], LN_KSCALE)

            # --- PE p-state warm-up while DMAs land ---
            warm_ps = pr_pool.tile([4, JC], f32, tag="psr", name="warm_ps")
            wst = cpool.tile([128, 2, 16], fp8)
            nc.vector.memset(wst[:], 0.0)
            wk = cpool.tile([128, 2, JC], fp8)
            nc.vector.memset(wk[:], 0.0)
            for _ in range(16):
                nc.tensor.matmul(warm_ps[:], wst[:, :, 0:4], wk[:],
                                 start=True, stop=True, perf_mode=DR)

            # --- main pipeline: columns in order, r-matmuls one column behind
            pending = []   # (o, s, kbuf, psr_t)
            done = [0]

            def flush(o, s, kbuf, psr_t):
                npair = s // 2
                for p in range(npair):
                    g = base_o[o] + 2 * p
                    nc.tensor.matmul(
                        psr_t[:], st_t[:, g:g + 2, 0:4], kbuf[:, 2 * p:2 * p + 2, :],
                        start=(p == 0), stop=(p == npair - 1 and s % 2 == 0),
                        perf_mode=DR)
                if s % 2:
                    g = base_o[o] + s - 1
                    nc.tensor.matmul(
                        psr_t[:], st_t[:, g, 0:4], kbuf[:, s - 1, :],
                        start=(s == 1), stop=True)
                rs_t = rpool.tile([4, JC], f32, tag="rs", name=f"rs{o}")
                nc.vector.tensor_mul(rs_t[:], psr_t[:],
                                     wj_t[:, o * JC:(o + 1) * JC])
                nc.vector.tensor_reduce(stage_t[:, o:o + 1], rs_t[:],
                                        axis=mybir.AxisListType.X,
                                        op=mybir.AluOpType.add)
                done[0] += 1
                if done[0] == 14:
                    nc.sync.dma_start(stage_d.ap()[:, 0:14], stage_t[:, 0:14])

            for o in range(N_COLS):
                s = S_O[o]
                kbuf = kpool.tile([128, 5, JC], fp8, tag="kg", name=f"kb{o}")
                psr_t = pr_pool.tile([4, JC], f32, tag="psr", name=f"pr{o}")
                for g0 in range(0, s, 3):
                    ln = min(3, s - g0)
                    ps = pe_pool.tile([128, 3, JC], f32, tag="pse")
                    for u in range(ln):
                        gslot = base_o[o] + g0 + u
                        nc.tensor.matmul(
                            ps[:, u, :],
                            uf_t[:, :, gslot * IT:(gslot + 1) * IT],
                            vf_t[:, :, o * JC:(o + 1) * JC],
                            start=True, stop=True, perf_mode=DR)
                    nc.scalar.activation(
                        kbuf[:, g0:g0 + ln, :], ps[:, 0:ln, :],
                        mybir.ActivationFunctionType.Exp,
                        scale=-1.0, bias=bias_t[:, 0:1])
                    if g0 == 0:
                        nc.vector.tensor_mul(kbuf[:, 0, :], kbuf[:, 0, :],
                                             mk_t[:])
                pending.append((o, s, kbuf, psr_t))
                if len(pending) > 1:
                    flush(*pending.pop(0))
            while pending:
                flush(*pending.pop(0))

            nc.sync.dma_start(stage_d.ap()[:, 14:], stage_t[:, 14:])

    nc.compile()
    return nc


# ---------------- host-side features ----------------

def _f8(v):
    return np.asarray(np.asarray(v).astype(E4), np.float64)


def _features(probs, image):
    ys, xs = np.meshgrid(np.arange(H, dtype=np.float64),
                         np.arange(W, dtype=np.float64), indexing="ij")
    y = ys.ravel()[PERM]
    x = xs.ravel()[PERM]
    col = image[0].astype(np.float64).reshape(3, N)[:, PERM]
    a = probs[0, 0].astype(np.float64).reshape(N)[PERM]
    b = 1.0 - a

    rC1 = np.sqrt(C1)
    yt, xt, gt = rC1 * y, rC1 * x, (rC1 * LAM) * col
    base = yt * yt + xt * xt + (gt * gt).sum(axis=0)
    B1 = _f8(base); B2 = _f8(base - B1); B3 = _f8(base - B1 - B2)
    one = np.ones(N)
    U, V = [], []
    for t in (B1, B2, B3):
        U.append(t); V.append(one)
    for t in (B1, B2, B3):
        U.append(one); V.append(t)

    def cross(w):
        h = _f8(w); r = w - h; m = _f8(r); l = _f8(r - m)
        for ui, vj in [(h, h), (h, m), (m, h), (h, l), (l, h), (m, m)]:
            U.append(_f8(-2.0 * ui)); V.append(vj)

    cross(yt); cross(xt)
    for ch in range(3):
        cross(gt[ch])
    U = np.stack(U).astype(E4)      # [36, N]
    V = np.stack(V).astype(E4)

    ah = _f8(a); al = _f8(a - ah); bh = _f8(b); bl = _f8(b - bh)
    stat = np.stack([ah, al, bh, bl], axis=1).astype(E4)   # [N, 4]
    diag = float((a * b).sum())
    return U, V, stat, a, b, diag


def kernel(probs: np.ndarray, image: np.ndarray) -> np.ndarray:
    probs = np.asarray(probs)
    image = np.asarray(image)
    assert probs.shape == (1, 2, H, W) and image.shape == (1, 3, H, W)

    if "nc" not in _CACHE:
        _CACHE["nc"] = _build_program()
    nc = _CACHE["nc"]

    U, V, stat, a, b, diag = _features(probs, image)
    U2 = U.reshape(2, RHALF, N)     # rank k = i*18 + p
    V2 = V.reshape(2, RHALF, N)

    vf = np.zeros((RHALF, 2, N), dtype=E4)
    wj = np.zeros((4, N), dtype=BF)
    for o, m in enumerate(COL_ORDER):
        cols = slice(m * JC, (m + 1) * JC)
        ocols = slice(o * JC, (o + 1) * JC)
        vf[:, 0, ocols] = V2[0][:, cols]
        vf[:, 1, ocols] = V2[1][:, cols]
        wj[0, ocols] = wj[1, ocols] = b[cols]
        wj[2, ocols] = wj[3, ocols] = a[cols]

    in_maps = []
    for c in range(N_CORES):
        uf = np.zeros((RHALF, 2, NSLOTS * IT), dtype=E4)
        st = np.zeros((128, NSLOTS, 16), dtype=E4)
        g = 0
        for o, m in enumerate(COL_ORDER):
            for s in range(S_O[o]):
                t = ASSIGN[m][c, s]
                if t >= 0:
                    iw = slice(t * IT, (t + 1) * IT)
                    uf[:, 0, g * IT:(g + 1) * IT] = U2[0][:, iw]
                    uf[:, 1, g * IT:(g + 1) * IT] = U2[1][:, iw]
                    st[:, g, 0:4] = stat[iw, :]
                g += 1
        if c < 4:
            th = (128 * c + np.arange(128, dtype=np.float32)).reshape(128, 1)
        else:
            th = np.full((128, 1), -(10 ** 6), dtype=np.float32)
        in_maps.append({"uf": uf, "vf": vf, "st": st, "th": th, "wj": wj})
    _CACHE["in_maps"] = in_maps

    res = run_bass_kernel_spmd(nc, in_maps, list(range(N_CORES)))
    tri = np.float64(0.0)
    for c in range(N_CORES):
        tri += res.results[c]["stage"].astype(np.float64).sum()

    loss = 2.0 * (tri / KSCALE + diag) / N
    return np.float32(loss)


# revision 10
# speedup vs baseline: 1.5394x; 1.5191x over previous
"""Dense CRF pairwise loss on 8 Trainium2 NeuronCores.

loss = (2/N) * [ sum_{i<j} (a_i b_j + a_j b_i) K_ij + sum_i a_i b_i ],
a = probs[:,0], b = 1-a, K_ij = exp(-c1*d_xy - c2*d_rgb), K_ii = 1.

Pixels are permuted into 8x16 patches (i-tiles of 128) grouped into 16x32
chunks (j-columns of 512).  The device computes the strictly-off-diagonal
upper-triangle blocks (i-tile t vs chunk m, t < 4m) whose patch boxes are
within RCUT=33 px (the Gaussian tail beyond is negligible): 55 slots/core.
The 18 block-diagonal 512x512 triangles and the K_ii=1 diagonal are done
on host in numpy (~3% of pairs) -- no masks needed on device.

The exponent x = c1*d_xy + c2*d_rgb is ONE fp8e4m3 matmul per block:
features are pre-scaled by sqrt(c1) and hi/mid/lo-split so every value is
e4m3-exact; rank 36, zero-padded to a 96-partition contraction because the
PE runs 2 cols/cycle only when the contraction dim is >= ~96 (measured:
K=96 -> 242 ns per 512-col matmul, K<=88 -> 443 ns).  ScalarE applies
exp(-x + ln 128) writing K~ = 128*K in fp8 (the 2^7 scale preserves
small-K mass against the e4m3 subnormal floor).

Per column, per-slot stats [ah al bh bl] (fp8, 16B-padded) reduce K~ over
i with fp8 DoubleRow r-matmuls, two slots per matmul ([128,2,4]^T @
[128,2,512] -> [4,512], 248 ns = 2 fp8 cols/cycle); odd tails use a plain
fp8 matmul.  DVE copies each column's [4,512] PSUM accumulator to a stage
buffer; the stage is DMAd out and dotted with [b,b,a,a] on host.

SPMD: all cores run the identical 55-slot schedule; slot -> i-tile is
per-core data.  Padding slots have zero features and zero stats
(exp -> 128, stats 0 -> no contribution).
"""

import numpy as np
import ml_dtypes

import concourse.bass as bass
import concourse.tile as tile
from concourse import bacc, mybir
from concourse.bass_utils import run_bass_kernel_spmd

E4 = ml_dtypes.float8_e4m3

H = W = 96
N = H * W                       # 9216
N_CORES = 8
JC = 512                        # column width (one 16x32 chunk)
N_CHUNKS = 18
IT = 128                        # i-tile (8x16 patch)
KPART = 96                      # matmul contraction partitions (36 real)
RANK = 36
RCUT = 33.0                     # patch-box cull radius (px)
KSCALE = 128.0                  # K stored as 128*K in fp8
LN_KSCALE = float(np.log(KSCALE))

SIGMA_XY = 15.0
SIGMA_RGB = 0.125
C1 = 1.0 / (2.0 * SIGMA_XY * SIGMA_XY)
C2 = 1.0 / (2.0 * SIGMA_RGB * SIGMA_RGB)
LAM = np.sqrt(C2 / C1)          # 120

_CACHE = {}


# ---------------- geometry: patches, chunks, cull, schedule ----------------

def _geometry():
    boxes = []          # per patch (y0,y1,x0,x1) inclusive
    perm = []           # new pixel index -> original row-major index
    for cy in range(6):
        for cx in range(3):
            for py in range(2):
                for px in range(2):
                    y0, x0 = cy * 16 + py * 8, cx * 32 + px * 16
                    boxes.append((y0, y0 + 7, x0, x0 + 15))
                    for yy in range(y0, y0 + 8):
                        for xx in range(x0, x0 + 16):
                            perm.append(yy * 96 + xx)
    perm = np.array(perm)
    cbox = [(min(boxes[4 * m + q][0] for q in range(4)),
             max(boxes[4 * m + q][1] for q in range(4)),
             min(boxes[4 * m + q][2] for q in range(4)),
             max(boxes[4 * m + q][3] for q in range(4))) for m in range(N_CHUNKS)]

    def bdist(b1, b2):
        dy = max(0, b1[0] - b2[1], b2[0] - b1[1])
        dx = max(0, b1[2] - b2[3], b2[2] - b1[3])
        return float(np.hypot(dy, dx))

    # strictly-off-diagonal kept blocks only (t < 4m); diagonal on host
    kept = {m: [t for t in range(4 * m)
                if bdist(boxes[t], cbox[m]) <= RCUT] for m in range(N_CHUNKS)}
    cols = [m for m in range(N_CHUNKS) if kept[m]]
    s_m = {m: -(-len(kept[m]) // N_CORES) for m in cols}
    # big columns first (DMA streaming + short tail), small columns last
    col_order = sorted(cols, key=lambda m: -s_m[m])
    s_o = [s_m[m] for m in col_order]

    assign = {}
    for m in cols:
        A = -np.ones((N_CORES, s_m[m]), int)
        for k, t in enumerate(kept[m]):
            A[k % N_CORES, k // N_CORES] = t
        assign[m] = A
    return perm, col_order, s_o, assign


PERM, COL_ORDER, S_O, ASSIGN = _geometry()
N_COLS = len(COL_ORDER)         # 17
NSLOTS = sum(S_O)               # 55


# ---------------- device program ----------------

def _build_program():
    nc = bacc.Bacc("TRN2", target_bir_lowering=False, debug=False)
    f32 = mybir.dt.float32
    fp8 = mybir.dt.float8e4
    DR = mybir.MatmulPerfMode.DoubleRow

    uf_d = nc.dram_tensor("uf", [KPART, NSLOTS * IT], fp8, kind="ExternalInput")
    vf_d = nc.dram_tensor("vf", [KPART, N], fp8, kind="ExternalInput")
    st_d = nc.dram_tensor("st", [128, NSLOTS, 16], fp8, kind="ExternalInput")
    stage_d = nc.dram_tensor("stage", [4, N_COLS * JC], f32, kind="ExternalOutput")

    base_o = np.concatenate([[0], np.cumsum(S_O)]).astype(int)

    with tile.TileContext(nc) as tc:
        with (
            tc.tile_pool(name="const", bufs=1) as cpool,
            tc.tile_pool(name="kgrp", bufs=2) as kpool,
            tc.tile_pool(name="pse", bufs=2, space="PSUM") as pe_pool,
            tc.tile_pool(name="psr", bufs=2, space="PSUM") as pr_pool,
        ):
            uf_t = cpool.tile([KPART, NSLOTS * IT], fp8)
            vf_t = cpool.tile([KPART, N], fp8)
            st_t = cpool.tile([128, NSLOTS, 16], fp8)
            stage_t = cpool.tile([4, N_COLS * JC], f32)
            warm_t = cpool.tile([KPART, JC], fp8)
            bias_t = cpool.tile([128, 1], f32)

            # --- input DMAs (sync/gpsimd/scalar queues), first-needed first
            nc.gpsimd.dma_start(uf_t[:, 0:10 * IT], uf_d.ap()[:, 0:10 * IT])
            nc.gpsimd.dma_start(uf_t[:, 10 * IT:30 * IT],
                                uf_d.ap()[:, 10 * IT:30 * IT])
            nc.gpsimd.dma_start(uf_t[:, 30 * IT:], uf_d.ap()[:, 30 * IT:])
            nc.sync.dma_start(st_t[:], st_d.ap())
            nc.sync.dma_start(vf_t[:, 0:3 * JC], vf_d.ap()[:, 0:3 * JC])
            nc.sync.dma_start(vf_t[:, 3 * JC:9 * JC], vf_d.ap()[:, 3 * JC:9 * JC])
            nc.scalar.dma_start(vf_t[:, 9 * JC:], vf_d.ap()[:, 9 * JC:])
            nc.vector.memset(warm_t[:], 0.0)
            nc.vector.memset(bias_t[:], LN_KSCALE)

            # --- PE warm-up while DMAs land ---
            warm_ps = pe_pool.tile([128, 3, JC], f32, tag="pse", name="warm_ps")
            for _ in range(10):
                nc.tensor.matmul(warm_ps[:, 0, :], warm_t[:, 0:IT], warm_t[:],
                                 start=True, stop=True)

            # --- main pipeline: columns in order, r-matmuls one column behind
            pending = []
            done = [0]

            def flush(o, s, kbuf, psr_t):
                npair = s // 2
                for p in range(npair):
                    g = base_o[o] + 2 * p
                    nc.tensor.matmul(
                        psr_t[:], st_t[:, g:g + 2, 0:4], kbuf[:, 2 * p:2 * p + 2, :],
                        start=(p == 0), stop=(p == npair - 1 and s % 2 == 0),
                        perf_mode=DR)
                if s % 2:
                    g = base_o[o] + s - 1
                    nc.tensor.matmul(
                        psr_t[:], st_t[:, g, 0:4], kbuf[:, s - 1, :],
                        start=(s == 1), stop=True)
                nc.vector.tensor_copy(stage_t[:, o * JC:(o + 1) * JC], psr_t[:])
                done[0] += 1
                if done[0] == 7:
                    nc.sync.dma_start(stage_d.ap()[:, 0:7 * JC],
                                      stage_t[:, 0:7 * JC])
                elif done[0] == 13:
                    nc.gpsimd.dma_start(stage_d.ap()[:, 7 * JC:13 * JC],
                                        stage_t[:, 7 * JC:13 * JC])

            for o in range(N_COLS):
                s = S_O[o]
                kbuf = kpool.tile([128, 5, JC], fp8, tag="kg", name=f"kb{o}")
                psr_t = pr_pool.tile([4, JC], f32, tag="psr", name=f"pr{o}")
                for g0 in range(0, s, 3):
                    ln = min(3, s - g0)
                    ps = pe_pool.tile([128, 3, JC], f32, tag="pse")
                    for u in range(ln):
                        gslot = base_o[o] + g0 + u
                        nc.tensor.matmul(
                            ps[:, u, :],
                            uf_t[:, gslot * IT:(gslot + 1) * IT],
                            vf_t[:, o * JC:(o + 1) * JC],
                            start=True, stop=True)
                    nc.scalar.activation(
                        kbuf[:, g0:g0 + ln, :], ps[:, 0:ln, :],
                        mybir.ActivationFunctionType.Exp,
                        scale=-1.0, bias=bias_t[:, 0:1])
                pending.append((o, s, kbuf, psr_t))
                if len(pending) > 1:
                    flush(*pending.pop(0))
            while pending:
                flush(*pending.pop(0))

            nc.sync.dma_start(stage_d.ap()[:, 13 * JC:], stage_t[:, 13 * JC:])

    nc.compile()
    return nc


# ---------------- host-side features ----------------

def _f8(v):
    return np.asarray(np.asarray(v).astype(E4), np.float64)


def _features(probs, image):
    ys, xs = np.meshgrid(np.arange(H, dtype=np.float64),
                         np.arange(W, dtype=np.float64), indexing="ij")
    y = ys.ravel()[PERM]
    x = xs.ravel()[PERM]
    col = image[0].astype(np.float64).reshape(3, N)[:, PERM]
    a = probs[0, 0].astype(np.float64).reshape(N)[PERM]
    b = 1.0 - a

    rC1 = np.sqrt(C1)
    yt, xt, gt = rC1 * y, rC1 * x, (rC1 * LAM) * col
    base = yt * yt + xt * xt + (gt * gt).sum(axis=0)
    B1 = _f8(base); B2 = _f8(base - B1); B3 = _f8(base - B1 - B2)
    one = np.ones(N)
    U, V = [], []
    for t in (B1, B2, B3):
        U.append(t); V.append(one)
    for t in (B1, B2, B3):
        U.append(one); V.append(t)

    def cross(w):
        h = _f8(w); r = w - h; m = _f8(r); l = _f8(r - m)
        for ui, vj in [(h, h), (h, m), (m, h), (h, l), (l, h), (m, m)]:
            U.append(_f8(-2.0 * ui)); V.append(vj)

    cross(yt); cross(xt)
    for ch in range(3):
        cross(gt[ch])
    U = np.stack(U).astype(E4)      # [36, N]
    V = np.stack(V).astype(E4)

    ah = _f8(a); al = _f8(a - ah); bh = _f8(b); bl = _f8(b - bh)
    stat = np.stack([ah, al, bh, bl], axis=1).astype(E4)   # [N, 4]
    return U, V, stat, a, b, y, x, col


def _host_diag(y, x, col, a, b):
    """K_ii diagonal plus the 18 in-chunk 512x512 upper triangles (fp64)."""
    tot = float((a * b).sum())
    iu = np.triu_indices(JC, k=1)
    for m in range(N_CHUNKS):
        sl = slice(m * JC, (m + 1) * JC)
        yy, xx, aa, bb = y[sl], x[sl], a[sl], b[sl]
        cc = col[:, sl]
        dxy = (yy[:, None] - yy[None, :]) ** 2 + (xx[:, None] - xx[None, :]) ** 2
        drgb = ((cc[:, :, None] - cc[:, None, :]) ** 2).sum(axis=0)
        K = np.exp(-C1 * dxy - C2 * drgb)
        w = aa[:, None] * bb[None, :] + bb[:, None] * aa[None, :]
        tot += float((w[iu] * K[iu]).sum())
    return tot


def kernel(probs: np.ndarray, image: np.ndarray) -> np.ndarray:
    probs = np.asarray(probs)
    image = np.asarray(image)
    assert probs.shape == (1, 2, H, W) and image.shape == (1, 3, H, W)

    if "nc" not in _CACHE:
        _CACHE["nc"] = _build_program()
    nc = _CACHE["nc"]

    U, V, stat, a, b, y, x, col = _features(probs, image)

    vf = np.zeros((KPART, N), dtype=E4)
    for o, m in enumerate(COL_ORDER):
        vf[:RANK, o * JC:(o + 1) * JC] = V[:, m * JC:(m + 1) * JC]

    in_maps = []
    for c in range(N_CORES):
        uf = np.zeros((KPART, NSLOTS * IT), dtype=E4)
        st = np.zeros((128, NSLOTS, 16), dtype=E4)
        g = 0
        for o, m in enumerate(COL_ORDER):
            for s in range(S_O[o]):
                t = ASSIGN[m][c, s]
                if t >= 0:
                    iw = slice(t * IT, (t + 1) * IT)
                    uf[:RANK, g * IT:(g + 1) * IT] = U[:, iw]
                    st[:, g, 0:4] = stat[iw, :]
                g += 1
        in_maps.append({"uf": uf, "vf": vf, "st": st})
    _CACHE["in_maps"] = in_maps

    res = run_bass_kernel_spmd(nc, in_maps, list(range(N_CORES)))

    tri = np.float64(0.0)
    for c in range(N_CORES):
        stage = res.results[c]["stage"].astype(np.float64)   # [4, 17*512]
        for o, m in enumerate(COL_ORDER):
            jw = slice(m * JC, (m + 1) * JC)
            r = stage[:, o * JC:(o + 1) * JC]
            tri += ((r[0] + r[1]) * b[jw]).sum() + ((r[2] + r[3]) * a[jw]).sum()

    tri /= KSCALE
    tri += _host_diag(y, x, col, a, b)
    loss = 2.0 * tri / N
    return np.float32(loss)
